# revision 1
# baseline (speedup 1.0000x reference)
"""L1 loss (mean |yhat - y|) over (64, 128, 4096) fp32 tensors on 8 TRN2 cores.

Strategy: pure data-parallel over the batch dim. Core i takes batch rows
[8i, 8i+8) of both tensors. The host interleaves yhat/y per tile into one
DRAM tensor z so each [128 x 8192] SBUF tile loads with a single 4 MiB DMA
(32 KB contiguous per partition; also keeps every compute instruction at
<=1 semaphore wait, a DVE ISA limit). Per tile the vector engine computes
d = yhat - y and a per-partition abs-sum reduce into one column of a
[128, 8] accumulator, which is DMA'd out. The host sums all partials in
float64 and divides by the global element count.
"""

import numpy as np

import concourse.bacc as bacc
import concourse.bass as bass
import concourse.mybir as mybir
import concourse.tile as tile
from concourse.bass_utils import run_bass_kernel_spmd

N_CORES = 8
FULL_SHAPE = (64, 128, 4096)
TOTAL_ELEMS = FULL_SHAPE[0] * FULL_SHAPE[1] * FULL_SHAPE[2]  # 33,554,432

P = 128                                  # SBUF partitions
ELEMS_PER_CORE = TOTAL_ELEMS // N_CORES  # 4,194,304 per input tensor
F_TILE = 4096                            # floats per partition per tensor per tile
N_TILES = ELEMS_PER_CORE // (P * F_TILE) # 8 tiles; 2*F_TILE*128*4B = 4 MiB per DMA

_nc_cache = []


def _build_nc():
    # Bacc (not raw Bass): its compile() pipeline runs
    # generate_event_semaphores, which splits multi-wait sync_infos to
    # satisfy the TRN2 1-wait-per-instruction constraint walrus enforces.
    nc = bacc.Bacc("TRN2", target_bir_lowering=False, debug=False)
    z = nc.declare_dram_parameter(
        "z", [N_TILES, P, 2 * F_TILE], mybir.dt.float32, isOutput=False
    )
    out = nc.declare_dram_parameter("out", [P, N_TILES], mybir.dt.float32, isOutput=True)

    with tile.TileContext(nc) as tc:
        with (
            tc.tile_pool(name="io", bufs=3) as io_pool,
            tc.tile_pool(name="diff", bufs=2) as diff_pool,
            tc.tile_pool(name="acc", bufs=1) as acc_pool,
        ):
            acc = acc_pool.tile([P, N_TILES], mybir.dt.float32)
            for i in range(N_TILES):
                zt = io_pool.tile([P, 2 * F_TILE], mybir.dt.float32, tag="z")
                nc.sync.dma_start(zt[:], z[i])
                d = diff_pool.tile([P, F_TILE], mybir.dt.float32, tag="d")
                nc.vector.tensor_sub(d[:], zt[:, 0:F_TILE], zt[:, F_TILE : 2 * F_TILE])
                nc.vector.tensor_reduce(
                    acc[:, i : i + 1],
                    d[:],
                    axis=mybir.AxisListType.X,
                    op=mybir.AluOpType.add,
                    apply_absolute_value=True,
                )
            nc.sync.dma_start(out[:], acc[:])
    nc.compile()
    return nc


def _get_nc():
    if not _nc_cache:
        _nc_cache.append(_build_nc())
    return _nc_cache[0]


def _shard_inputs(yhat: np.ndarray, y: np.ndarray) -> list[dict[str, np.ndarray]]:
    yhat_t = np.ascontiguousarray(yhat, dtype=np.float32).reshape(
        N_CORES, N_TILES, P, F_TILE
    )
    y_t = np.ascontiguousarray(y, dtype=np.float32).reshape(
        N_CORES, N_TILES, P, F_TILE
    )
    z = np.empty((N_CORES, N_TILES, P, 2, F_TILE), dtype=np.float32)
    z[:, :, :, 0, :] = yhat_t
    z[:, :, :, 1, :] = y_t
    z = z.reshape(N_CORES, N_TILES, P, 2 * F_TILE)
    return [{"z": z[c]} for c in range(N_CORES)]


def kernel(yhat: np.ndarray, y: np.ndarray) -> np.ndarray:
    nc = _get_nc()
    in_maps = _shard_inputs(yhat, y)
    res = run_bass_kernel_spmd(nc, in_maps, list(range(N_CORES)))
    total = np.float64(0.0)
    for r in res.results:
        total += r["out"].astype(np.float64).sum()
    return np.asarray(total / TOTAL_ELEMS, dtype=np.float32)

